# revision 48
# baseline (speedup 1.0000x reference)
"""Trainium2 Bass kernel for DifferentialAttention (B=2, S=2048, DIM=2048).

Sharding: 8 cores = 2 batches x 4 head-groups (4 heads each). Per core:
  QKV projection (column-parallel) + RoPE, differential attention for 4
  heads, row-parallel wo partial; host sums 4 partials per batch.

v2 design (driven by the TimelineSim cost model, where matmul cost =
output-free-size only and PE clock ramps only while continuously busy):
  * pv matmuls flipped to [queries, dv] orientation with a ones column in
    V: the softmax denominator accumulates for free in the same PSUM tile
    (baseline spent ~55us of PE streaming ones-contractions).
  * RMSNorm done with per-partition-scalar DVE ops (queries on
    partitions), division-free:
       u'' = d2*pv1 - (lam*d1)*pv2 = d1*d2*a
       out = u'' * rsqrt(mean(u''^2) + EPS*(d1*d2)^2)
    rsqrt via Ln/Exp (exp table also hosts ln/square/copy: no act-table
    switches). This kills all broadcast/mean matmuls of the baseline.
  * RoPE pair-swap via a 128x128 permutation matmul on the PE (free-size
    512 each) instead of 48 cross-partition SBUF DMAs.
  * normed attn transposed back to [dims, s] via PE transposes (64 x 128
    cycles) to feed the row-parallel wo matmul.
  * weights repacked host-side so every DMA descriptor is >=512B;
    output stores issued from the (otherwise idle) gpsimd queue.

Layouts (partition dim first):
  QT [128,4,S], KT [128,2,S]: rows [E0 O0 E1 O1] x32 (evens-first perm).
  Vn [128,16,2,129]: s-tile major, per kv head 128 v-dims + ones col.
  et [128,1024]: exp(scores) with keys on partitions.
  ppv12 [128,258]: pv+denom for both branches of one (head, q-tile).
  ustT [128,4,S]: normed attn, wo lhsT layout.
"""

import math
import numpy as np
import ml_dtypes
import concourse.bass as bass
import concourse.tile as tile
from concourse import bacc, mybir
from concourse.bass_utils import run_bass_kernel_spmd
from contextlib import ExitStack

F32 = mybir.dt.float32
BF16 = mybir.dt.bfloat16
AF = mybir.ActivationFunctionType
ALU = mybir.AluOpType

DIM = 2048
S = 2048
HD = 64
EPS = 1e-5
SCALE = HD ** -0.5
NCORES = 8
B = 2

W = 512                 # stage-A s-chunk width
ET_DT = mybir.dt.bfloat16   # exp(scores) storage
ET_BUFS = 40 if ET_DT == mybir.dt.float8e4 else 22
PRE_H2 = ET_DT == mybir.dt.float8e4   # h2 exp precompute needs 32+ live tiles
OUT_DT = mybir.dt.bfloat16
TRACE = False
DEBUG = False           # add intermediate-dump outputs
LAST_RESULTS = None


# ---------------------------------------------------------------- device program

def build_program(lam: float):
    nc = bacc.Bacc("TRN2", target_bir_lowering=False, debug=False,
                   num_devices=NCORES)
    io = {}
    for name, shape, d in [
        ("xT", [DIM, S], BF16),
        ("wqk_s", [6, 128, 2048], BF16),
        ("wv_s", [DIM, 256], BF16),
        ("wo_s", [512, DIM], BF16),
        ("cs128", [128, S], BF16), ("sn128", [128, S], BF16),
        ("P128", [128, 128], BF16), ("ident", [128, 128], BF16),
    ]:
        io[name] = nc.dram_tensor(name, shape, d, kind="ExternalInput").ap()
    out = nc.dram_tensor("out", [S, DIM], OUT_DT, kind="ExternalOutput").ap()
    dbg = {}
    if DEBUG:
        for name, shape, d in [
            ("qtd", [128, 4, S], BF16), ("ktd", [128, 2, S], BF16),
            ("vnd", [128, 16, 2, 129], BF16), ("etd", [128, 1024], F32),
            ("ustd", [128, 4, S], BF16), ("rsd", [128, 4, 8], F32),
            ("ddd", [128, 4, 8], F32), ("msd", [128, 4, 8], F32),
            ("wqkd", [128, 6, 16, 128], BF16), ("xhd", [128, 16, W], BF16),
            ("csd", [128, S], BF16), ("p128d", [128, 128], BF16),
        ]:
            dbg[name] = nc.dram_tensor(name, shape, d,
                                       kind="ExternalOutput").ap()

    with tile.TileContext(nc) as tc:
        _body(tc, io, out, lam, dbg)
    nc.compile()
    return nc


def _body(tc, io, out, lam, dbg=None):
    nc = tc.nc
    with ExitStack() as top:
        stash = top.enter_context(tc.tile_pool(name="stash", bufs=1))
        QT = stash.tile([128, 4, S], BF16)
        KT = stash.tile([128, 2, S], BF16)
        Vn = stash.tile([128, 16, 2, 129], BF16)
        cs = stash.tile([128, S], BF16)
        sn = stash.tile([128, S], BF16)
        P128t = stash.tile([128, 128], BF16)
        identt = stash.tile([128, 128], BF16)
        wqk = stash.tile([128, 6, 16, 128], BF16)
        wv_all = stash.tile([128, 16, 256], BF16)
        wot = stash.tile([128, 4, S], BF16)
        ustT = stash.tile([128, 4, S], BF16)

        # exp shift: fp8e4m3 saturates to NaN above 448, so compute
        # exp(s*scale - 2); the shift cancels in the softmax ratio
        ebias = stash.tile([128, 1], F32)
        nc.vector.memset(ebias[:], -2.0)

        # loads (SP queue); first-needed first: wqk0 + first x chunk gate
        # the first matmul, cs/sn gate the first RoPE
        wqk4 = io["wqk_s"].rearrange("t p (a c) -> t p a c", a=16)
        nc.sync.dma_start(wqk[:, 0], wqk4[0])
        nc.vector.memset(Vn[:, :, :, 128:129], 1.0)

        with ExitStack() as run:
            xp = run.enter_context(tc.tile_pool(name="xh", bufs=2))
            rtmp = run.enter_context(tc.tile_pool(name="ropetmp", bufs=2))
            ep = run.enter_context(tc.tile_pool(name="expt", bufs=ET_BUFS))
            cpool = run.enter_context(tc.tile_pool(name="cpool", bufs=3))
            upool = run.enter_context(tc.tile_pool(name="upool", bufs=34))
            spool = run.enter_context(tc.tile_pool(name="stats", bufs=2))
            obp = run.enter_context(tc.tile_pool(name="obp", bufs=2))
            ps_sc = run.enter_context(
                tc.tile_pool(name="ps_sc", bufs=2, space="PSUM"))

            ets = {}   # (h, j) -> list of 8 et tiles (current qh)

            # ---- stage A with early-scores interleave
            xT3 = io["xT"].rearrange("(a p) s -> p a s", p=128)
            with ExitStack() as ac:
                ps_qs = ac.enter_context(
                    tc.tile_pool(name="ps_qs", bufs=3, space="PSUM"))
                ps_v = ac.enter_context(
                    tc.tile_pool(name="ps_v", bufs=1, space="PSUM"))
                def _first_chunk_loads():
                    # every load a stage-A instruction reads must be emitted
                    # BEFORE that instruction in trace order (Tile tracks RAW
                    # by trace position); order here = DMA service order
                    nc.sync.dma_start(cs[:], io["cs128"][:])
                    nc.sync.dma_start(sn[:], io["sn128"][:])

                def _post_x_loads():
                    nc.sync.dma_start(P128t[:], io["P128"][:])
                    for ct in range(1, 6):
                        nc.sync.dma_start(wqk[:, ct], wqk4[ct])
                    nc.sync.dma_start(
                        wv_all[:],
                        io["wv_s"].rearrange("(a p) c -> p a c", p=128))

                for sq in range(4):
                    _stage_a_chunk(tc, io, xT3, xp, rtmp, ps_qs, ps_v,
                                   wqk, wv_all, cs, sn, P128t, QT, KT, Vn, sq,
                                   hook=_first_chunk_loads if sq == 0 else None,
                                   hook2=_post_x_loads if sq == 0 else None,
                                   dbg=dbg)
                    if sq == 2:
                        # keys 0:1024 + queries of qh0 are ready: warm up the
                        # exp pipeline for h=0 while the PE chews on sq=3
                        nc.sync.dma_start(identt[:], io["ident"][:])
                        nc.sync.dma_start(
                            wot[:],
                            io["wo_s"].rearrange("(a p) c -> p a c", p=128))
                        for j in range(2):
                            _scores_exp(tc, ps_sc, ep, ets, QT, KT,
                                        ebias, qh=0, h=0, j=j)
                if PRE_H2:
                    # h=2 also attends keys 0:1023 -> its exps overlap the
                    # stage-A tail and the first pv phase
                    for j in range(2):
                        _scores_exp(tc, ps_sc, ep, ets, QT, KT, ebias, qh=0,
                                    h=2, j=j)

            with ExitStack() as bc:
                ps_pv = bc.enter_context(
                    tc.tile_pool(name="ps_pv", bufs=2, space="PSUM"))
                ps_tr = bc.enter_context(
                    tc.tile_pool(name="ps_tr", bufs=1, space="PSUM"))
                ps_o = bc.enter_context(
                    tc.tile_pool(name="ps_o", bufs=1, space="PSUM"))

                for qh in range(2):
                    # stats laid out [128, grp=h%2, idx=h//2, qt] so each
                    # head-group's rsqrt batches contiguously
                    ms = spool.tile([128, 2, 2, 8], F32, tag="ms")
                    dd = spool.tile([128, 2, 2, 8], F32, tag="dd")
                    rs = spool.tile([128, 2, 2, 8], F32, tag="rs")
                    us = {}

                    def _rs_all():
                        # rs = rsqrt(sum(u''^2) + 128*EPS*dd^2), one Ln/Exp
                        # site per query-half to minimize act-table loads
                        t32 = cpool.tile([128, 2, 2, 8], F32, tag="t16")
                        nc.vector.tensor_mul(t32[:], dd[:], dd[:])
                        nc.vector.scalar_tensor_tensor(
                            t32[:], t32[:], float(128.0 * EPS), ms[:],
                            op0=ALU.mult, op1=ALU.add)
                        nc.scalar.activation(t32[:], t32[:], AF.Ln,
                                             bias=0.0, scale=1.0)
                        nc.scalar.activation(rs[:], t32[:], AF.Exp,
                                             bias=0.0, scale=-0.5)

                    # software pipeline: pair each head's (exp-paced) score
                    # phase with the previous head's dense pv chains so the
                    # in-order PE queue never drains
                    if qh == 0 and PRE_H2:  # h0/h2 exps done in stage A
                        pairs = [(1, 0), (3, 2), (None, 1), (None, 3)]
                    elif qh == 0:           # h0 exps done in stage A
                        pairs = [(2, 0), (1, 2), (3, 1), (None, 3)]
                    else:
                        pairs = [(0, None), (2, 0), (1, 2), (3, 1), (None, 3)]
                    for sc_h, pv_h in pairs:
                        for kt in range(8):
                            if sc_h is not None:
                                for j in range(2):
                                    _scores_exp_tile(tc, ps_sc, ep, ets,
                                                     QT, KT, ebias, qh,
                                                     sc_h, j, kt)
                            if pv_h is not None:
                                _pv_qt(tc, ps_pv, cpool, upool, ets, Vn,
                                       us, ms, dd, pv_h, kt, lam)
                    _rs_all()
                    # tail: pass2 + wo interleaved per s-tile
                    for st in range(9):
                        if st < 8:
                            for h in (0, 2, 1, 3):
                                _norm_col(tc, ps_tr, cpool, us, rs, identt,
                                          ustT, qh, h, st)
                        if st > 0:
                            _wo_st(tc, ps_o, obp, ustT, wot, out, qh, st - 1)
                    if dbg and qh == 0:
                        nc.sync.dma_start(dbg["rsd"][:], rs[:])
                        nc.sync.dma_start(dbg["ddd"][:], dd[:])
                        nc.sync.dma_start(dbg["msd"][:], ms[:])
                        et8 = ep.tile([128, 1024], F32, tag="etdump", bufs=1)
                        nc.vector.tensor_copy(et8[:], ets[(0, 0)][0][:])
                        nc.sync.dma_start(dbg["etd"][:], et8[:])
            if dbg:
                nc.sync.dma_start(dbg["wqkd"][:], wqk[:])
                nc.sync.dma_start(dbg["csd"][:], cs[:])
                nc.sync.dma_start(dbg["p128d"][:], P128t[:])
                nc.sync.dma_start(dbg["qtd"][:], QT[:])
                nc.sync.dma_start(dbg["ktd"][:], KT[:])
                nc.sync.dma_start(dbg["vnd"][:], Vn[:])
                nc.sync.dma_start(dbg["ustd"][:], ustT[:])


def _stage_a_chunk(tc, io, xT3, xp, rtmp, ps_qs, ps_v, wqk, wv_all,
                   cs, sn, P128t, QT, KT, Vn, sq, hook=None, hook2=None,
                   dbg=None):
    """QKV projection + RoPE for s-chunk [sq*512, sq*512+512)."""
    nc = tc.nc
    ssl = slice(sq * W, sq * W + W)
    xh = xp.tile([128, 16, W], BF16, tag="xh")
    if hook is not None:
        nc.sync.dma_start(xh[:, 0:4, :], xT3[:, 0:4, ssl])
        hook()
        for c in range(1, 4):
            nc.sync.dma_start(xh[:, 4 * c:4 * c + 4, :],
                              xT3[:, 4 * c:4 * c + 4, ssl])
    else:
        for c in range(2):
            nc.sync.dma_start(xh[:, 8 * c:8 * c + 8, :],
                              xT3[:, 8 * c:8 * c + 8, ssl])
    if hook2 is not None:
        hook2()
    if dbg and sq == 3:
        nc.sync.dma_start(dbg["xhd"][:], xh[:])
    for ct in range(6):
        dest, di = (QT, ct) if ct < 4 else (KT, ct - 4)
        pq = ps_qs.tile([128, W], F32, tag="qs", name=f"pq{sq}_{ct}")
        for dt_i in range(16):
            nc.tensor.matmul(pq[:], lhsT=wqk[:, ct, dt_i, :],
                             rhs=xh[:, dt_i, :],
                             start=(dt_i == 0), stop=(dt_i == 15))
        # RoPE: newE = E*c + swap(O*-s); newO = O*c + swap(E*s)
        t2 = rtmp.tile([128, W], BF16, tag="t2")
        nc.vector.tensor_mul(t2[:], pq[:], sn[:, ssl])
        pswp = ps_qs.tile([128, W], F32, tag="qs", name=f"sw{sq}_{ct}")
        nc.tensor.matmul(pswp[:], lhsT=P128t[:], rhs=t2[:],
                         start=True, stop=True)
        t1 = rtmp.tile([128, W], F32, tag="t1")
        nc.vector.tensor_mul(t1[:], pq[:], cs[:, ssl])
        nc.vector.tensor_add(dest[:, di, ssl], t1[:], pswp[:])
    for st in range(4):
        psv = ps_v.tile([128, 2, 128], F32, tag="psv")
        for dt_i in range(16):
            nc.tensor.matmul(psv[:], lhsT=xh[:, dt_i, st * 128:st * 128 + 128],
                             rhs=wv_all[:, dt_i, :],
                             start=(dt_i == 0), stop=(dt_i == 15))
        nc.vector.tensor_copy(Vn[:, sq * 4 + st, :, 0:128], psv[:])


def _scores_exp_tile(tc, ps_sc, ep, ets, QT, KT, ebias, qh, h, j, kt):
    """scores^T (keys on partitions) -> exp, one 128-key tile."""
    nc = tc.nc
    kvl, rho = h // 2, h % 2
    kof = rho * 1024 + kt * 128
    psc = ps_sc.tile([128, 1024], F32, tag="sc")
    for nch in range(2):
        nsl = slice(nch * 512, nch * 512 + 512)
        qsl = slice(qh * 1024 + nch * 512, qh * 1024 + nch * 512 + 512)
        nc.tensor.matmul(psc[:, nsl],
                         lhsT=KT[j * 64:(j + 1) * 64, kvl, kof:kof + 128],
                         rhs=QT[j * 64:(j + 1) * 64, h, qsl],
                         start=True, stop=True)
    et = ep.tile([128, 1024], ET_DT, tag="e")
    nc.scalar.activation(et[:], psc[:], AF.Exp, bias=ebias[:, 0:1],
                         scale=float(SCALE))
    ets.setdefault((h, j), [None] * 8)[kt] = et


def _scores_exp(tc, ps_sc, ep, ets, QT, KT, ebias, qh, h, j):
    for kt in range(8):
        _scores_exp_tile(tc, ps_sc, ep, ets, QT, KT, ebias, qh, h, j, kt)


def _pv_qt(tc, ps_pv, cpool, upool, ets, Vn, us, ms, dd, h, qt, lam):
    """pv both branches for one q-tile; u'' = pv1*d2 - lam*d1*pv2 + stats."""
    nc = tc.nc
    kvl, rho = h // 2, h % 2
    qsl = slice(qt * 128, qt * 128 + 128)
    ppv = ps_pv.tile([128, 2, 256], F32, tag="pv")  # bank-aligned halves
    for j in range(2):
        ej = ets[(h, j)]
        for kt in range(8):
            nc.tensor.matmul(ppv[:, j, 0:129],
                             lhsT=ej[kt][:, qsl],
                             rhs=Vn[:, rho * 8 + kt, kvl, :],
                             start=(kt == 0), stop=(kt == 7))
    # w2 = pv2 * (lam*d1);  dd = d1*d2;  u'' = pv1*d2 - w2
    w2 = cpool.tile([128, 128], F32, tag="w2")
    nc.vector.tensor_scalar(w2[:], ppv[:, 1, 0:128], ppv[:, 0, 128:129],
                            float(lam), op0=ALU.mult, op1=ALU.mult)
    nc.vector.tensor_scalar(dd[:, h % 2, h // 2, qt:qt + 1],
                            ppv[:, 0, 128:129], ppv[:, 1, 128:129],
                            None, op0=ALU.mult)
    u2 = upool.tile([128, 128], BF16, tag="u")
    nc.vector.scalar_tensor_tensor(u2[:], ppv[:, 0, 0:128],
                                   ppv[:, 1, 128:129], w2[:],
                                   op0=ALU.mult, op1=ALU.subtract)
    usq = cpool.tile([128, 128], BF16, tag="usq")
    nc.vector.tensor_mul(usq[:], u2[:], u2[:])
    nc.vector.tensor_reduce(ms[:, h % 2, h // 2, qt:qt + 1], usq[:],
                            op=ALU.add, axis=mybir.AxisListType.X)
    us[(h, qt)] = u2


def _norm_col(tc, ps_tr, cpool, us, rs, identt, ustT, qh, h, qt):
    """ust = u'' * rs * sqrt(128); transpose into wo-lhsT layout."""
    nc = tc.nc
    # sqrt(128) restores the mean-vs-sum normalization of x'
    ust = cpool.tile([128, 128], BF16, tag="ust")
    nc.vector.tensor_scalar(ust[:], us[(h, qt)][:],
                            rs[:, h % 2, h // 2, qt:qt + 1],
                            float(math.sqrt(128.0)),
                            op0=ALU.mult, op1=ALU.mult)
    pst = ps_tr.tile([128, 128], BF16, tag="tr")
    nc.tensor.transpose(pst[:], ust[:], identt[:])
    nc.vector.tensor_copy(ustT[:, h, qh * 1024 + qt * 128:
                               qh * 1024 + qt * 128 + 128], pst[:])


def _wo_st(tc, ps_o, obp, ustT, wot, out, qh, st):
    """row-parallel wo for one s-tile; single batched bf16 store."""
    nc = tc.nc
    sof = (qh * 8 + st) * 128
    ob = obp.tile([128, 2048], OUT_DT, tag="ob")
    for ech in range(4):
        po = ps_o.tile([128, 512], F32, tag="po")
        for r in range(4):
            nc.tensor.matmul(po[:], lhsT=ustT[:, r, sof:sof + 128],
                             rhs=wot[:, r, ech * 512:ech * 512 + 512],
                             start=(r == 0), stop=(r == 3))
        osl = slice(ech * 512, ech * 512 + 512)
        if ech % 2 == 0:
            nc.scalar.activation(ob[:, osl], po[:], AF.Copy, bias=0.0,
                                 scale=1.0)
        else:
            nc.vector.tensor_copy(ob[:, osl], po[:])
    nc.sync.dma_start(out[sof:sof + 128, :], ob[:])


# ---------------------------------------------------------------- host side

_PERM64 = np.concatenate([np.arange(0, 64, 2), np.arange(1, 64, 2)])


def make_core_inputs(core, x, wq, wk, wv, wo, subln_w, lambda_init,
                     freqs_cos, freqs_sin):
    b, g = divmod(core, 4)
    bf = ml_dtypes.bfloat16
    qcols = np.empty(512, np.int64)
    for hl in range(4):
        for j in range(2):
            qcols[hl * 128 + j * 64:hl * 128 + j * 64 + 64] = \
                ((4 * g + hl) * 2 + j) * 64 + _PERM64
    kcols = np.empty(256, np.int64)
    for kvl in range(2):
        for j in range(2):
            kcols[kvl * 128 + j * 64:kvl * 128 + j * 64 + 64] = \
                ((2 * g + kvl) * 2 + j) * 64 + _PERM64
    vcols = np.arange(256) + 2 * g * 128

    # wq/wk packed per column-tile: [6, 128(part), 16*128] so every DMA
    # descriptor is one contiguous 4KB run per partition.
    wq_c = wq[:, qcols].astype(np.float32)   # [2048, 512]
    wk_c = wk[:, kcols].astype(np.float32)   # [2048, 256]
    wqk = np.empty((6, 128, 2048), np.float32)
    for ct in range(4):
        wqk[ct] = wq_c[:, ct * 128:(ct + 1) * 128].reshape(
            16, 128, 128).transpose(1, 0, 2).reshape(128, 2048)
    for ct in range(2):
        wqk[4 + ct] = wk_c[:, ct * 128:(ct + 1) * 128].reshape(
            16, 128, 128).transpose(1, 0, 2).reshape(128, 2048)

    cosT = np.ascontiguousarray(freqs_cos.T.astype(np.float32))  # [32, S]
    sinT = np.ascontiguousarray(freqs_sin.T.astype(np.float32))
    wo_s = wo[512 * g: 512 * g + 512, :].astype(np.float32).copy()
    wo_s *= np.tile(subln_w.astype(np.float32)
                    * (1.0 - np.float32(np.asarray(lambda_init)[0])), 4)[:, None]

    swap = np.empty(128, np.int64)
    for blk in range(4):
        swap[blk * 32:blk * 32 + 32] = \
            (blk + 1 if blk % 2 == 0 else blk - 1) * 32 + np.arange(32)
    P = np.zeros((128, 128), np.float32)
    P[swap, np.arange(128)] = 1.0   # P[p, r] = 1 iff p == swap(r)

    return {
        "xT": np.ascontiguousarray(x[b].T.astype(np.float32)).astype(bf),
        "wqk_s": wqk.astype(bf),
        "wv_s": np.ascontiguousarray(wv[:, vcols].astype(np.float32)).astype(bf),
        "wo_s": wo_s.astype(bf),
        "cs128": np.tile(cosT, (4, 1)).astype(bf),
        "sn128": np.concatenate([sinT, -sinT, sinT, -sinT], axis=0).astype(bf),
        "P128": P.astype(bf),
        "ident": np.eye(128, dtype=np.float32).astype(bf),
    }


def compute_lambda(lambda_q1, lambda_k1, lambda_q2, lambda_k2, lambda_init):
    l1 = np.exp(np.sum(np.float32(lambda_q1) * np.float32(lambda_k1),
                       dtype=np.float32))
    l2 = np.exp(np.sum(np.float32(lambda_q2) * np.float32(lambda_k2),
                       dtype=np.float32))
    return float(l1 - l2 + np.float32(np.asarray(lambda_init)[0]))


def kernel(x, wq, wk, wv, wo, lambda_q1, lambda_k1, lambda_q2, lambda_k2,
           lambda_init, subln_w, freqs_cos, freqs_sin):
    global LAST_RESULTS
    x = np.asarray(x); wq = np.asarray(wq); wk = np.asarray(wk)
    wv = np.asarray(wv); wo = np.asarray(wo)
    lam = compute_lambda(lambda_q1, lambda_k1, lambda_q2, lambda_k2, lambda_init)

    nc = build_program(lam)
    in_maps = [make_core_inputs(c, x, wq, wk, wv, wo,
                                np.asarray(subln_w), np.asarray(lambda_init),
                                np.asarray(freqs_cos), np.asarray(freqs_sin))
               for c in range(NCORES)]
    res = run_bass_kernel_spmd(nc, in_maps, list(range(NCORES)), trace=TRACE)
    LAST_RESULTS = res
    outs = [np.asarray(res.results[c]["out"]).astype(np.float32)
            for c in range(NCORES)]
    full = np.empty((B, S, DIM), np.float32)
    for b in range(B):
        full[b] = outs[4 * b] + outs[4 * b + 1] + outs[4 * b + 2] + outs[4 * b + 3]
    return full


# revision 52
# speedup vs baseline: 1.0639x; 1.0639x over previous
"""Trainium2 Bass kernel for DifferentialAttention (B=2, S=2048, DIM=2048).

Sharding: 8 cores = 2 batches x 4 head-groups (4 heads each). Per core:
  QKV projection (column-parallel) + RoPE, differential attention for 4
  heads, row-parallel wo partial; host sums 4 partials per batch.

v2 design (driven by the TimelineSim cost model, where matmul cost =
output-free-size only and PE clock ramps only while continuously busy):
  * pv matmuls flipped to [queries, dv] orientation with a ones column in
    V: the softmax denominator accumulates for free in the same PSUM tile
    (baseline spent ~55us of PE streaming ones-contractions).
  * RMSNorm done with per-partition-scalar DVE ops (queries on
    partitions), division-free:
       u'' = d2*pv1 - (lam*d1)*pv2 = d1*d2*a
       out = u'' * rsqrt(mean(u''^2) + EPS*(d1*d2)^2)
    rsqrt via Ln/Exp (exp table also hosts ln/square/copy: no act-table
    switches). This kills all broadcast/mean matmuls of the baseline.
  * RoPE pair-swap via a 128x128 permutation matmul on the PE (free-size
    512 each) instead of 48 cross-partition SBUF DMAs.
  * normed attn transposed back to [dims, s] via PE transposes (64 x 128
    cycles) to feed the row-parallel wo matmul.
  * weights repacked host-side so every DMA descriptor is >=512B;
    output stores issued from the (otherwise idle) gpsimd queue.

Layouts (partition dim first):
  QT [128,4,S], KT [128,2,S]: rows [E0 O0 E1 O1] x32 (evens-first perm).
  Vn [128,16,2,129]: s-tile major, per kv head 128 v-dims + ones col.
  et [128,1024]: exp(scores) with keys on partitions.
  ppv12 [128,258]: pv+denom for both branches of one (head, q-tile).
  ustT [128,4,S]: normed attn, wo lhsT layout.
"""

import math
import numpy as np
import ml_dtypes
import concourse.bass as bass
import concourse.tile as tile
from concourse import bacc, mybir
from concourse.bass_utils import run_bass_kernel_spmd
from contextlib import ExitStack

F32 = mybir.dt.float32
BF16 = mybir.dt.bfloat16
AF = mybir.ActivationFunctionType
ALU = mybir.AluOpType

DIM = 2048
S = 2048
HD = 64
EPS = 1e-5
SCALE = HD ** -0.5
NCORES = 8
B = 2

W = 512                 # stage-A s-chunk width
ET_DT = mybir.dt.bfloat16   # exp(scores) storage
ET_BUFS = 40 if ET_DT == mybir.dt.float8e4 else 22
PRE_H2 = ET_DT == mybir.dt.float8e4   # h2 exp precompute needs 32+ live tiles
OUT_DT = mybir.dt.bfloat16
TRACE = False
DEBUG = False           # add intermediate-dump outputs
LAST_RESULTS = None


# ---------------------------------------------------------------- device program

def build_program(lam: float):
    nc = bacc.Bacc("TRN2", target_bir_lowering=False, debug=False,
                   num_devices=NCORES)
    io = {}
    for name, shape, d in [
        ("xT", [DIM, S], BF16),
        ("wqk_s", [6, 128, 2048], BF16),
        ("wv_s", [DIM, 256], BF16),
        ("wo_s", [512, DIM], BF16),
        ("cs128", [128, S], BF16), ("sn128", [128, S], BF16),
        ("P128", [128, 128], BF16), ("ident", [128, 128], BF16),
    ]:
        io[name] = nc.dram_tensor(name, shape, d, kind="ExternalInput").ap()
    out = nc.dram_tensor("out", [S, DIM], OUT_DT, kind="ExternalOutput").ap()
    dbg = {}
    if DEBUG:
        for name, shape, d in [
            ("qtd", [128, 4, S], BF16), ("ktd", [128, 2, S], BF16),
            ("vnd", [128, 16, 2, 129], BF16), ("etd", [128, 1024], F32),
            ("ustd", [128, 4, S], BF16), ("rsd", [128, 4, 8], F32),
            ("ddd", [128, 4, 8], F32), ("msd", [128, 4, 8], F32),
            ("wqkd", [128, 6, 16, 128], BF16), ("xhd", [128, 16, W], BF16),
            ("csd", [128, S], BF16), ("p128d", [128, 128], BF16),
        ]:
            dbg[name] = nc.dram_tensor(name, shape, d,
                                       kind="ExternalOutput").ap()

    with tile.TileContext(nc) as tc:
        _body(tc, io, out, lam, dbg)
    nc.compile()
    return nc


def _body(tc, io, out, lam, dbg=None):
    nc = tc.nc
    with ExitStack() as top:
        stash = top.enter_context(tc.tile_pool(name="stash", bufs=1))
        QT = stash.tile([128, 4, S], BF16)
        KT = stash.tile([128, 2, S], BF16)
        Vn = stash.tile([128, 16, 2, 129], BF16)
        cs = stash.tile([128, S], BF16)
        sn = stash.tile([128, S], BF16)
        P128t = stash.tile([128, 128], BF16)
        identt = stash.tile([128, 128], BF16)
        wqk = stash.tile([128, 6, 16, 128], BF16)
        wv_all = stash.tile([128, 16, 256], BF16)
        wot = stash.tile([128, 4, S], BF16)
        ustT = stash.tile([128, 4, S], BF16)

        # exp shift: fp8e4m3 saturates to NaN above 448, so compute
        # exp(s*scale - 2); the shift cancels in the softmax ratio
        ebias = stash.tile([128, 1], F32)
        nc.vector.memset(ebias[:], -2.0)

        # loads (SP queue); first-needed first: wqk0 + first x chunk gate
        # the first matmul, cs/sn gate the first RoPE
        wqk4 = io["wqk_s"].rearrange("t p (a c) -> t p a c", a=16)
        nc.sync.dma_start(wqk[:, 0], wqk4[0])
        nc.vector.memset(Vn[:, :, :, 128:129], 1.0)

        with ExitStack() as run:
            xp = run.enter_context(tc.tile_pool(name="xh", bufs=2))
            rtmp = run.enter_context(tc.tile_pool(name="ropetmp", bufs=2))
            ep = run.enter_context(tc.tile_pool(name="expt", bufs=ET_BUFS))
            cpool = run.enter_context(tc.tile_pool(name="cpool", bufs=3))
            upool = run.enter_context(tc.tile_pool(name="upool", bufs=34))
            spool = run.enter_context(tc.tile_pool(name="stats", bufs=2))
            obp = run.enter_context(tc.tile_pool(name="obp", bufs=2))
            ps_sc = run.enter_context(
                tc.tile_pool(name="ps_sc", bufs=2, space="PSUM"))

            ets = {}   # (h, j) -> list of 8 et tiles (current qh)

            # ---- stage A with early-scores interleave
            xT3 = io["xT"].rearrange("(a p) s -> p a s", p=128)
            with ExitStack() as ac:
                ps_qs = ac.enter_context(
                    tc.tile_pool(name="ps_qs", bufs=3, space="PSUM"))
                ps_v = ac.enter_context(
                    tc.tile_pool(name="ps_v", bufs=1, space="PSUM"))
                def _first_chunk_loads():
                    # every load a stage-A instruction reads must be emitted
                    # BEFORE that instruction in trace order (Tile tracks RAW
                    # by trace position); order here = DMA service order
                    nc.sync.dma_start(cs[:], io["cs128"][:])
                    nc.sync.dma_start(sn[:], io["sn128"][:])

                def _post_x_loads():
                    nc.sync.dma_start(P128t[:], io["P128"][:])
                    for ct in range(1, 6):
                        nc.sync.dma_start(wqk[:, ct], wqk4[ct])
                    nc.sync.dma_start(
                        wv_all[:],
                        io["wv_s"].rearrange("(a p) c -> p a c", p=128))

                for sq in range(4):
                    _stage_a_chunk(tc, io, xT3, xp, rtmp, ps_qs, ps_v,
                                   wqk, wv_all, cs, sn, P128t, QT, KT, Vn, sq,
                                   hook=_first_chunk_loads if sq == 0 else None,
                                   hook2=_post_x_loads if sq == 0 else None,
                                   dbg=dbg)
                    if sq == 2:
                        # keys 0:1024 + queries of qh0 are ready: warm up the
                        # exp pipeline for h=0 while the PE chews on sq=3
                        nc.sync.dma_start(identt[:], io["ident"][:])
                        nc.sync.dma_start(
                            wot[:],
                            io["wo_s"].rearrange("(a p) c -> p a c", p=128))
                        for j in range(2):
                            _scores_exp(tc, ps_sc, ep, ets, QT, KT,
                                        ebias, qh=0, h=0, j=j)
                if PRE_H2:
                    # h=2 also attends keys 0:1023 -> its exps overlap the
                    # stage-A tail and the first pv phase
                    for j in range(2):
                        _scores_exp(tc, ps_sc, ep, ets, QT, KT, ebias, qh=0,
                                    h=2, j=j)

            with ExitStack() as bc:
                ps_pv = bc.enter_context(
                    tc.tile_pool(name="ps_pv", bufs=2, space="PSUM"))
                ps_tr = bc.enter_context(
                    tc.tile_pool(name="ps_tr", bufs=2, space="PSUM"))
                ps_o = ps_pv   # wo tiles reuse the pv slots (idle in tail)

                for qh in range(2):
                    # stats laid out [128, grp=h%2, idx=h//2, qt] so each
                    # head-group's rsqrt batches contiguously
                    ms = spool.tile([128, 2, 2, 8], F32, tag="ms")
                    dd = spool.tile([128, 2, 2, 8], F32, tag="dd")
                    rs = spool.tile([128, 2, 2, 8], F32, tag="rs")
                    us = {}

                    def _rs_all():
                        # rs = rsqrt(sum(u''^2) + 128*EPS*dd^2), one Ln/Exp
                        # site per query-half to minimize act-table loads
                        t32 = cpool.tile([128, 2, 2, 8], F32, tag="t16")
                        nc.vector.tensor_mul(t32[:], dd[:], dd[:])
                        nc.vector.scalar_tensor_tensor(
                            t32[:], t32[:], float(128.0 * EPS), ms[:],
                            op0=ALU.mult, op1=ALU.add)
                        nc.scalar.activation(t32[:], t32[:], AF.Ln,
                                             bias=0.0, scale=1.0)
                        nc.scalar.activation(rs[:], t32[:], AF.Exp,
                                             bias=0.0, scale=-0.5)

                    # software pipeline: pair each head's (exp-paced) score
                    # phase with the previous head's dense pv chains so the
                    # in-order PE queue never drains
                    if qh == 0 and PRE_H2:  # h0/h2 exps done in stage A
                        pairs = [(1, 0), (3, 2), (None, 1), (None, 3)]
                    elif qh == 0:           # h0 exps done in stage A
                        pairs = [(2, 0), (1, 2), (3, 1), (None, 3)]
                    else:   # qh1 h0 exps were sandwiched into qh0's tail
                        pairs = [(2, 0), (1, 2), (3, 1), (None, 3)]
                    for sc_h, pv_h in pairs:
                        for kt in range(8):
                            if sc_h is not None:
                                for j in range(2):
                                    _scores_exp_tile(tc, ps_sc, ep, ets,
                                                     QT, KT, ebias, qh,
                                                     sc_h, j, kt)
                            if pv_h is not None:
                                _pv_qt(tc, ps_pv, cpool, upool, ets, Vn,
                                       us, ms, dd, pv_h, kt, lam)
                    _rs_all()
                    # tail: pass2 + wo interleaved per s-tile; for qh0,
                    # also warm qh1's first exp pipeline inside the tail so
                    # the qh transition never drains the PE
                    for st in range(9):
                        if st < 8:
                            for h in (0, 2, 1, 3):
                                _norm_col(tc, ps_tr, cpool, us, rs, identt,
                                          ustT, qh, h, st)
                            if qh == 0:
                                for j in range(2):
                                    _scores_exp_tile(tc, ps_sc, ep, ets,
                                                     QT, KT, ebias, 1, 0,
                                                     j, st)
                        if st > 0:
                            _wo_st(tc, ps_o, obp, ustT, wot, out, qh, st - 1)
                    if dbg and qh == 0:
                        nc.sync.dma_start(dbg["rsd"][:], rs[:])
                        nc.sync.dma_start(dbg["ddd"][:], dd[:])
                        nc.sync.dma_start(dbg["msd"][:], ms[:])
                        et8 = ep.tile([128, 1024], F32, tag="etdump", bufs=1)
                        nc.vector.tensor_copy(et8[:], ets[(0, 0)][0][:])
                        nc.sync.dma_start(dbg["etd"][:], et8[:])
            if dbg:
                nc.sync.dma_start(dbg["wqkd"][:], wqk[:])
                nc.sync.dma_start(dbg["csd"][:], cs[:])
                nc.sync.dma_start(dbg["p128d"][:], P128t[:])
                nc.sync.dma_start(dbg["qtd"][:], QT[:])
                nc.sync.dma_start(dbg["ktd"][:], KT[:])
                nc.sync.dma_start(dbg["vnd"][:], Vn[:])
                nc.sync.dma_start(dbg["ustd"][:], ustT[:])


def _stage_a_chunk(tc, io, xT3, xp, rtmp, ps_qs, ps_v, wqk, wv_all,
                   cs, sn, P128t, QT, KT, Vn, sq, hook=None, hook2=None,
                   dbg=None):
    """QKV projection + RoPE for s-chunk [sq*512, sq*512+512)."""
    nc = tc.nc
    ssl = slice(sq * W, sq * W + W)
    xh = xp.tile([128, 16, W], BF16, tag="xh")
    if hook is not None:
        nc.sync.dma_start(xh[:, 0:4, :], xT3[:, 0:4, ssl])
        hook()
        for c in range(1, 4):
            nc.sync.dma_start(xh[:, 4 * c:4 * c + 4, :],
                              xT3[:, 4 * c:4 * c + 4, ssl])
    else:
        for c in range(2):
            nc.sync.dma_start(xh[:, 8 * c:8 * c + 8, :],
                              xT3[:, 8 * c:8 * c + 8, ssl])
    if hook2 is not None:
        hook2()
    if dbg and sq == 3:
        nc.sync.dma_start(dbg["xhd"][:], xh[:])
    for ct in range(6):
        dest, di = (QT, ct) if ct < 4 else (KT, ct - 4)
        pq = ps_qs.tile([128, W], F32, tag="qs", name=f"pq{sq}_{ct}")
        for dt_i in range(16):
            nc.tensor.matmul(pq[:], lhsT=wqk[:, ct, dt_i, :],
                             rhs=xh[:, dt_i, :],
                             start=(dt_i == 0), stop=(dt_i == 15))
        # RoPE: newE = E*c + swap(O*-s); newO = O*c + swap(E*s)
        t2 = rtmp.tile([128, W], BF16, tag="t2")
        nc.vector.tensor_mul(t2[:], pq[:], sn[:, ssl])
        pswp = ps_qs.tile([128, W], F32, tag="qs", name=f"sw{sq}_{ct}")
        nc.tensor.matmul(pswp[:], lhsT=P128t[:], rhs=t2[:],
                         start=True, stop=True)
        t1 = rtmp.tile([128, W], F32, tag="t1")
        nc.vector.tensor_mul(t1[:], pq[:], cs[:, ssl])
        nc.vector.tensor_add(dest[:, di, ssl], t1[:], pswp[:])
    for st in range(4):
        psv = ps_v.tile([128, 2, 128], F32, tag="psv")
        for dt_i in range(16):
            nc.tensor.matmul(psv[:], lhsT=xh[:, dt_i, st * 128:st * 128 + 128],
                             rhs=wv_all[:, dt_i, :],
                             start=(dt_i == 0), stop=(dt_i == 15))
        nc.vector.tensor_copy(Vn[:, sq * 4 + st, :, 0:128], psv[:])


def _scores_exp_tile(tc, ps_sc, ep, ets, QT, KT, ebias, qh, h, j, kt):
    """scores^T (keys on partitions) -> exp, one 128-key tile."""
    nc = tc.nc
    kvl, rho = h // 2, h % 2
    kof = rho * 1024 + kt * 128
    psc = ps_sc.tile([128, 1024], F32, tag="sc")
    for nch in range(2):
        nsl = slice(nch * 512, nch * 512 + 512)
        qsl = slice(qh * 1024 + nch * 512, qh * 1024 + nch * 512 + 512)
        nc.tensor.matmul(psc[:, nsl],
                         lhsT=KT[j * 64:(j + 1) * 64, kvl, kof:kof + 128],
                         rhs=QT[j * 64:(j + 1) * 64, h, qsl],
                         start=True, stop=True)
    et = ep.tile([128, 1024], ET_DT, tag="e")
    nc.scalar.activation(et[:], psc[:], AF.Exp, bias=ebias[:, 0:1],
                         scale=float(SCALE))
    ets.setdefault((h, j), [None] * 8)[kt] = et


def _scores_exp(tc, ps_sc, ep, ets, QT, KT, ebias, qh, h, j):
    for kt in range(8):
        _scores_exp_tile(tc, ps_sc, ep, ets, QT, KT, ebias, qh, h, j, kt)


def _pv_qt(tc, ps_pv, cpool, upool, ets, Vn, us, ms, dd, h, qt, lam):
    """pv both branches for one q-tile; u'' = pv1*d2 - lam*d1*pv2 + stats."""
    nc = tc.nc
    kvl, rho = h // 2, h % 2
    qsl = slice(qt * 128, qt * 128 + 128)
    ppv = ps_pv.tile([128, 2, 256], F32, tag="pv")  # bank-aligned halves
    for j in range(2):
        ej = ets[(h, j)]
        for kt in range(8):
            nc.tensor.matmul(ppv[:, j, 0:129],
                             lhsT=ej[kt][:, qsl],
                             rhs=Vn[:, rho * 8 + kt, kvl, :],
                             start=(kt == 0), stop=(kt == 7))
    # w2 = pv2 * (lam*d1);  dd = d1*d2;  u'' = pv1*d2 - w2
    w2 = cpool.tile([128, 128], F32, tag="w2")
    nc.vector.tensor_scalar(w2[:], ppv[:, 1, 0:128], ppv[:, 0, 128:129],
                            float(lam), op0=ALU.mult, op1=ALU.mult)
    nc.vector.tensor_scalar(dd[:, h % 2, h // 2, qt:qt + 1],
                            ppv[:, 0, 128:129], ppv[:, 1, 128:129],
                            None, op0=ALU.mult)
    u2 = upool.tile([128, 128], BF16, tag="u")
    nc.vector.scalar_tensor_tensor(u2[:], ppv[:, 0, 0:128],
                                   ppv[:, 1, 128:129], w2[:],
                                   op0=ALU.mult, op1=ALU.subtract)
    usq = cpool.tile([128, 128], BF16, tag="usq")
    nc.vector.tensor_mul(usq[:], u2[:], u2[:])
    nc.vector.tensor_reduce(ms[:, h % 2, h // 2, qt:qt + 1], usq[:],
                            op=ALU.add, axis=mybir.AxisListType.X)
    us[(h, qt)] = u2


def _norm_col(tc, ps_tr, cpool, us, rs, identt, ustT, qh, h, qt):
    """ust = u'' * rs * sqrt(128); transpose into wo-lhsT layout."""
    nc = tc.nc
    # sqrt(128) restores the mean-vs-sum normalization of x'
    ust = cpool.tile([128, 128], BF16, tag="ust")
    nc.vector.tensor_scalar(ust[:], us[(h, qt)][:],
                            rs[:, h % 2, h // 2, qt:qt + 1],
                            float(math.sqrt(128.0)),
                            op0=ALU.mult, op1=ALU.mult)
    pst = ps_tr.tile([128, 128], BF16, tag="tr")
    nc.tensor.transpose(pst[:], ust[:], identt[:])
    nc.vector.tensor_copy(ustT[:, h, qh * 1024 + qt * 128:
                               qh * 1024 + qt * 128 + 128], pst[:])


def _wo_st(tc, ps_o, obp, ustT, wot, out, qh, st):
    """row-parallel wo for one s-tile; single batched bf16 store."""
    nc = tc.nc
    sof = (qh * 8 + st) * 128
    ob = obp.tile([128, 2048], OUT_DT, tag="ob")
    for ech in range(4):
        po3 = ps_o.tile([128, 2, 256], F32, tag="pv", name="po")
        po = po3.rearrange("p a b -> p (a b)")
        for r in range(4):
            nc.tensor.matmul(po[:], lhsT=ustT[:, r, sof:sof + 128],
                             rhs=wot[:, r, ech * 512:ech * 512 + 512],
                             start=(r == 0), stop=(r == 3))
        osl = slice(ech * 512, ech * 512 + 512)
        if ech % 2 == 0:
            nc.scalar.activation(ob[:, osl], po[:], AF.Copy, bias=0.0,
                                 scale=1.0)
        else:
            nc.vector.tensor_copy(ob[:, osl], po[:])
    nc.sync.dma_start(out[sof:sof + 128, :], ob[:])


# ---------------------------------------------------------------- host side

_PERM64 = np.concatenate([np.arange(0, 64, 2), np.arange(1, 64, 2)])


def make_core_inputs(core, x, wq, wk, wv, wo, subln_w, lambda_init,
                     freqs_cos, freqs_sin):
    b, g = divmod(core, 4)
    bf = ml_dtypes.bfloat16
    qcols = np.empty(512, np.int64)
    for hl in range(4):
        for j in range(2):
            qcols[hl * 128 + j * 64:hl * 128 + j * 64 + 64] = \
                ((4 * g + hl) * 2 + j) * 64 + _PERM64
    kcols = np.empty(256, np.int64)
    for kvl in range(2):
        for j in range(2):
            kcols[kvl * 128 + j * 64:kvl * 128 + j * 64 + 64] = \
                ((2 * g + kvl) * 2 + j) * 64 + _PERM64
    vcols = np.arange(256) + 2 * g * 128

    # wq/wk packed per column-tile: [6, 128(part), 16*128] so every DMA
    # descriptor is one contiguous 4KB run per partition.
    wq_c = wq[:, qcols].astype(np.float32)   # [2048, 512]
    wk_c = wk[:, kcols].astype(np.float32)   # [2048, 256]
    wqk = np.empty((6, 128, 2048), np.float32)
    for ct in range(4):
        wqk[ct] = wq_c[:, ct * 128:(ct + 1) * 128].reshape(
            16, 128, 128).transpose(1, 0, 2).reshape(128, 2048)
    for ct in range(2):
        wqk[4 + ct] = wk_c[:, ct * 128:(ct + 1) * 128].reshape(
            16, 128, 128).transpose(1, 0, 2).reshape(128, 2048)

    cosT = np.ascontiguousarray(freqs_cos.T.astype(np.float32))  # [32, S]
    sinT = np.ascontiguousarray(freqs_sin.T.astype(np.float32))
    wo_s = wo[512 * g: 512 * g + 512, :].astype(np.float32).copy()
    wo_s *= np.tile(subln_w.astype(np.float32)
                    * (1.0 - np.float32(np.asarray(lambda_init)[0])), 4)[:, None]

    swap = np.empty(128, np.int64)
    for blk in range(4):
        swap[blk * 32:blk * 32 + 32] = \
            (blk + 1 if blk % 2 == 0 else blk - 1) * 32 + np.arange(32)
    P = np.zeros((128, 128), np.float32)
    P[swap, np.arange(128)] = 1.0   # P[p, r] = 1 iff p == swap(r)

    return {
        "xT": np.ascontiguousarray(x[b].T.astype(np.float32)).astype(bf),
        "wqk_s": wqk.astype(bf),
        "wv_s": np.ascontiguousarray(wv[:, vcols].astype(np.float32)).astype(bf),
        "wo_s": wo_s.astype(bf),
        "cs128": np.tile(cosT, (4, 1)).astype(bf),
        "sn128": np.concatenate([sinT, -sinT, sinT, -sinT], axis=0).astype(bf),
        "P128": P.astype(bf),
        "ident": np.eye(128, dtype=np.float32).astype(bf),
    }


def compute_lambda(lambda_q1, lambda_k1, lambda_q2, lambda_k2, lambda_init):
    l1 = np.exp(np.sum(np.float32(lambda_q1) * np.float32(lambda_k1),
                       dtype=np.float32))
    l2 = np.exp(np.sum(np.float32(lambda_q2) * np.float32(lambda_k2),
                       dtype=np.float32))
    return float(l1 - l2 + np.float32(np.asarray(lambda_init)[0]))


def kernel(x, wq, wk, wv, wo, lambda_q1, lambda_k1, lambda_q2, lambda_k2,
           lambda_init, subln_w, freqs_cos, freqs_sin):
    global LAST_RESULTS
    x = np.asarray(x); wq = np.asarray(wq); wk = np.asarray(wk)
    wv = np.asarray(wv); wo = np.asarray(wo)
    lam = compute_lambda(lambda_q1, lambda_k1, lambda_q2, lambda_k2, lambda_init)

    nc = build_program(lam)
    in_maps = [make_core_inputs(c, x, wq, wk, wv, wo,
                                np.asarray(subln_w), np.asarray(lambda_init),
                                np.asarray(freqs_cos), np.asarray(freqs_sin))
               for c in range(NCORES)]
    res = run_bass_kernel_spmd(nc, in_maps, list(range(NCORES)), trace=TRACE)
    LAST_RESULTS = res
    outs = [np.asarray(res.results[c]["out"]).astype(np.float32)
            for c in range(NCORES)]
    full = np.empty((B, S, DIM), np.float32)
    for b in range(B):
        full[b] = outs[4 * b] + outs[4 * b + 1] + outs[4 * b + 2] + outs[4 * b + 3]
    return full


# revision 55
# speedup vs baseline: 1.0762x; 1.0116x over previous
"""Trainium2 Bass kernel for DifferentialAttention (B=2, S=2048, DIM=2048).

Sharding: 8 cores = 2 batches x 4 head-groups (4 heads each). Per core:
  QKV projection (column-parallel) + RoPE, differential attention for 4
  heads, row-parallel wo partial; host sums 4 partials per batch.

v2 design (driven by the TimelineSim cost model, where matmul cost =
output-free-size only and PE clock ramps only while continuously busy):
  * pv matmuls flipped to [queries, dv] orientation with a ones column in
    V: the softmax denominator accumulates for free in the same PSUM tile
    (baseline spent ~55us of PE streaming ones-contractions).
  * RMSNorm done with per-partition-scalar DVE ops (queries on
    partitions), division-free:
       u'' = d2*pv1 - (lam*d1)*pv2 = d1*d2*a
       out = u'' * rsqrt(mean(u''^2) + EPS*(d1*d2)^2)
    rsqrt via Ln/Exp (exp table also hosts ln/square/copy: no act-table
    switches). This kills all broadcast/mean matmuls of the baseline.
  * RoPE pair-swap via a 128x128 permutation matmul on the PE (free-size
    512 each) instead of 48 cross-partition SBUF DMAs.
  * normed attn transposed back to [dims, s] via PE transposes (64 x 128
    cycles) to feed the row-parallel wo matmul.
  * weights repacked host-side so every DMA descriptor is >=512B;
    output stores issued from the (otherwise idle) gpsimd queue.

Layouts (partition dim first):
  QT [128,4,S], KT [128,2,S]: rows [E0 O0 E1 O1] x32 (evens-first perm).
  Vn [128,16,2,129]: s-tile major, per kv head 128 v-dims + ones col.
  et [128,1024]: exp(scores) with keys on partitions.
  ppv12 [128,258]: pv+denom for both branches of one (head, q-tile).
  ustT [128,4,S]: normed attn, wo lhsT layout.
"""

import math
import numpy as np
import ml_dtypes
import concourse.bass as bass
import concourse.tile as tile
from concourse import bacc, mybir
from concourse.bass_utils import run_bass_kernel_spmd
from contextlib import ExitStack

F32 = mybir.dt.float32
BF16 = mybir.dt.bfloat16
AF = mybir.ActivationFunctionType
ALU = mybir.AluOpType

DIM = 2048
S = 2048
HD = 64
EPS = 1e-5
SCALE = HD ** -0.5
NCORES = 8
B = 2

W = 512                 # stage-A s-chunk width
ET_DT = mybir.dt.bfloat16   # exp(scores) storage
ET_BUFS = 40 if ET_DT == mybir.dt.float8e4 else 22
PRE_H2 = ET_DT == mybir.dt.float8e4   # h2 exp precompute needs 32+ live tiles
OUT_DT = mybir.dt.bfloat16
TRACE = False
DEBUG = False           # add intermediate-dump outputs
LAST_RESULTS = None


# ---------------------------------------------------------------- device program

def build_program(lam: float):
    nc = bacc.Bacc("TRN2", target_bir_lowering=False, debug=False,
                   num_devices=NCORES)
    io = {}
    for name, shape, d in [
        ("xT", [DIM, S], BF16),
        ("wqk_s", [6, 128, 2048], BF16),
        ("wv_s", [DIM, 256], BF16),
        ("wo_s", [512, DIM], BF16),
        ("cs128", [128, S], BF16), ("sn128", [128, S], BF16),
        ("P128", [128, 128], BF16), ("ident", [128, 128], BF16),
    ]:
        io[name] = nc.dram_tensor(name, shape, d, kind="ExternalInput").ap()
    out = nc.dram_tensor("out", [S, DIM], OUT_DT, kind="ExternalOutput").ap()
    dbg = {}
    if DEBUG:
        for name, shape, d in [
            ("qtd", [128, 4, S], BF16), ("ktd", [128, 2, S], BF16),
            ("vnd", [128, 16, 2, 129], BF16), ("etd", [128, 1024], F32),
            ("ustd", [128, 4, S], BF16), ("rsd", [128, 4, 8], F32),
            ("ddd", [128, 4, 8], F32), ("msd", [128, 4, 8], F32),
            ("wqkd", [128, 6, 16, 128], BF16), ("xhd", [128, 16, W], BF16),
            ("csd", [128, S], BF16), ("p128d", [128, 128], BF16),
        ]:
            dbg[name] = nc.dram_tensor(name, shape, d,
                                       kind="ExternalOutput").ap()

    with tile.TileContext(nc) as tc:
        _body(tc, io, out, lam, dbg)
    nc.compile()
    return nc


def _body(tc, io, out, lam, dbg=None):
    nc = tc.nc
    with ExitStack() as top:
        stash = top.enter_context(tc.tile_pool(name="stash", bufs=1))
        QT = stash.tile([128, 4, S], BF16)
        KT = stash.tile([128, 2, S], BF16)
        Vn = stash.tile([128, 16, 2, 129], BF16)
        cs = stash.tile([128, S], BF16)
        sn = stash.tile([128, S], BF16)
        P128t = stash.tile([128, 128], BF16)
        identt = stash.tile([128, 128], BF16)
        wqk = stash.tile([128, 6, 16, 128], BF16)
        wv_all = stash.tile([128, 16, 256], BF16)
        wot = stash.tile([128, 4, S], BF16)
        ustT = stash.tile([128, 4, S], BF16)

        # exp shift: fp8e4m3 saturates to NaN above 448, so compute
        # exp(s*scale - 2); the shift cancels in the softmax ratio
        ebias = stash.tile([128, 1], F32)
        nc.vector.memset(ebias[:], -2.0)

        # loads (SP queue); first-needed first: wqk0 + first x chunk gate
        # the first matmul, cs/sn gate the first RoPE
        wqk4 = io["wqk_s"].rearrange("t p (a c) -> t p a c", a=16)
        nc.sync.dma_start(wqk[:, 0], wqk4[0])
        nc.vector.memset(Vn[:, :, :, 128:129], 1.0)

        with ExitStack() as run:
            xp = run.enter_context(tc.tile_pool(name="xh", bufs=2))
            rtmp = run.enter_context(tc.tile_pool(name="ropetmp", bufs=2))
            ep = run.enter_context(tc.tile_pool(name="expt", bufs=ET_BUFS))
            cpool = run.enter_context(tc.tile_pool(name="cpool", bufs=3))
            upool = run.enter_context(tc.tile_pool(name="upool", bufs=34))
            spool = run.enter_context(tc.tile_pool(name="stats", bufs=2))
            obp = run.enter_context(tc.tile_pool(name="obp", bufs=2))
            ps_sc = run.enter_context(
                tc.tile_pool(name="ps_sc", bufs=2, space="PSUM"))

            ets = {}   # (h, j) -> list of 8 et tiles (current qh)

            # ---- stage A with early-scores interleave
            xT3 = io["xT"].rearrange("(a p) s -> p a s", p=128)
            with ExitStack() as ac:
                ps_qs = ac.enter_context(
                    tc.tile_pool(name="ps_qs", bufs=3, space="PSUM"))
                ps_v = ac.enter_context(
                    tc.tile_pool(name="ps_v", bufs=1, space="PSUM"))
                def _first_chunk_loads():
                    # every load a stage-A instruction reads must be emitted
                    # BEFORE that instruction in trace order (Tile tracks RAW
                    # by trace position); order here = DMA service order
                    nc.sync.dma_start(cs[:], io["cs128"][:])
                    nc.sync.dma_start(sn[:], io["sn128"][:])

                def _post_x_loads():
                    nc.sync.dma_start(P128t[:], io["P128"][:])
                    for ct in range(1, 6):
                        nc.sync.dma_start(wqk[:, ct], wqk4[ct])
                    nc.sync.dma_start(
                        wv_all[:],
                        io["wv_s"].rearrange("(a p) c -> p a c", p=128))

                for sq in range(4):
                    _stage_a_chunk(tc, io, xT3, xp, rtmp, ps_qs, ps_v,
                                   wqk, wv_all, cs, sn, P128t, QT, KT, Vn, sq,
                                   hook=_first_chunk_loads if sq == 0 else None,
                                   hook2=_post_x_loads if sq == 0 else None,
                                   dbg=dbg)
                    if sq == 2:
                        # keys 0:1024 + queries of qh0 are ready: warm up the
                        # exp pipeline for h=0 while the PE chews on sq=3
                        nc.sync.dma_start(identt[:], io["ident"][:])
                        nc.sync.dma_start(
                            wot[:],
                            io["wo_s"].rearrange("(a p) c -> p a c", p=128))
                        for j in range(2):
                            _scores_exp(tc, ps_sc, ep, ets, QT, KT,
                                        ebias, qh=0, h=0, j=j)
                if PRE_H2:
                    # h=2 also attends keys 0:1023 -> its exps overlap the
                    # stage-A tail and the first pv phase
                    for j in range(2):
                        _scores_exp(tc, ps_sc, ep, ets, QT, KT, ebias, qh=0,
                                    h=2, j=j)

            with ExitStack() as bc:
                ps_pv = bc.enter_context(
                    tc.tile_pool(name="ps_pv", bufs=2, space="PSUM"))
                ps_tr = bc.enter_context(
                    tc.tile_pool(name="ps_tr", bufs=2, space="PSUM"))
                ps_o = ps_pv   # wo tiles reuse the pv slots (idle in tail)

                for qh in range(2):
                    # stats laid out [128, grp=h%2, idx=h//2, qt] so each
                    # head-group's rsqrt batches contiguously
                    ms = spool.tile([128, 2, 2, 8], F32, tag="ms")
                    dd = spool.tile([128, 2, 2, 8], F32, tag="dd")
                    rs = spool.tile([128, 2, 2, 8], F32, tag="rs")
                    us = {}

                    def _rs_all():
                        # rs = rsqrt(sum(u''^2) + 128*EPS*dd^2), one Ln/Exp
                        # site per query-half to minimize act-table loads
                        t32 = cpool.tile([128, 2, 2, 8], F32, tag="t16")
                        nc.vector.tensor_mul(t32[:], dd[:], dd[:])
                        nc.vector.scalar_tensor_tensor(
                            t32[:], t32[:], float(128.0 * EPS), ms[:],
                            op0=ALU.mult, op1=ALU.add)
                        nc.scalar.activation(t32[:], t32[:], AF.Ln,
                                             bias=0.0, scale=1.0)
                        nc.scalar.activation(rs[:], t32[:], AF.Exp,
                                             bias=0.0, scale=-0.5)

                    # software pipeline: pair each head's (exp-paced) score
                    # phase with the previous head's dense pv chains so the
                    # in-order PE queue never drains
                    if qh == 0 and PRE_H2:  # h0/h2 exps done in stage A
                        pairs = [(1, 0), (3, 2), (None, 1), (None, 3)]
                    elif qh == 0:           # h0 exps done in stage A
                        pairs = [(2, 0), (1, 2), (3, 1), (None, 3)]
                    else:   # qh1 h0 exps were sandwiched into qh0's tail
                        pairs = [(2, 0), (1, 2), (3, 1), (None, 3)]
                    for sc_h, pv_h in pairs:
                        for kt in range(8):
                            if sc_h is not None:
                                for j in range(2):
                                    _scores_exp_tile(tc, ps_sc, ep, ets,
                                                     QT, KT, ebias, qh,
                                                     sc_h, j, kt)
                            if pv_h is not None:
                                _pv_qt(tc, ps_pv, cpool, upool, ets, Vn,
                                       us, ms, dd, pv_h, kt, lam)
                    _rs_all()
                    # tail: pass2 + wo interleaved per s-tile; for qh0,
                    # also warm qh1's first exp pipeline inside the tail so
                    # the qh transition never drains the PE
                    for st in range(9):
                        if qh == 1 and st == 0:
                            # qh0's deferred last s-tile: ungated PE work
                            # that covers this qh's rsqrt-chain latency
                            _wo_st(tc, ps_o, obp, ustT, wot, out, 0, 7)
                        if st < 8:
                            if qh == 0:
                                # before the rs-gated norm cols so the PE
                                # stays busy through the rsqrt latency
                                for j in range(2):
                                    _scores_exp_tile(tc, ps_sc, ep, ets,
                                                     QT, KT, ebias, 1, 0,
                                                     j, st)
                            for h in (0, 2, 1, 3):
                                _norm_col(tc, ps_tr, cpool, us, rs, identt,
                                          ustT, qh, h, st)
                        if st > 0 and not (qh == 0 and st == 8):
                            _wo_st(tc, ps_o, obp, ustT, wot, out, qh, st - 1)
                    if dbg and qh == 0:
                        nc.sync.dma_start(dbg["rsd"][:], rs[:])
                        nc.sync.dma_start(dbg["ddd"][:], dd[:])
                        nc.sync.dma_start(dbg["msd"][:], ms[:])
                        et8 = ep.tile([128, 1024], F32, tag="etdump", bufs=1)
                        nc.vector.tensor_copy(et8[:], ets[(0, 0)][0][:])
                        nc.sync.dma_start(dbg["etd"][:], et8[:])
            if dbg:
                nc.sync.dma_start(dbg["wqkd"][:], wqk[:])
                nc.sync.dma_start(dbg["csd"][:], cs[:])
                nc.sync.dma_start(dbg["p128d"][:], P128t[:])
                nc.sync.dma_start(dbg["qtd"][:], QT[:])
                nc.sync.dma_start(dbg["ktd"][:], KT[:])
                nc.sync.dma_start(dbg["vnd"][:], Vn[:])
                nc.sync.dma_start(dbg["ustd"][:], ustT[:])


def _stage_a_chunk(tc, io, xT3, xp, rtmp, ps_qs, ps_v, wqk, wv_all,
                   cs, sn, P128t, QT, KT, Vn, sq, hook=None, hook2=None,
                   dbg=None):
    """QKV projection + RoPE for s-chunk [sq*512, sq*512+512)."""
    nc = tc.nc
    ssl = slice(sq * W, sq * W + W)
    xh = xp.tile([128, 16, W], BF16, tag="xh")
    if hook is not None:
        nc.sync.dma_start(xh[:, 0:4, :], xT3[:, 0:4, ssl])
        hook()
        for c in range(1, 4):
            nc.sync.dma_start(xh[:, 4 * c:4 * c + 4, :],
                              xT3[:, 4 * c:4 * c + 4, ssl])
    else:
        for c in range(2):
            nc.sync.dma_start(xh[:, 8 * c:8 * c + 8, :],
                              xT3[:, 8 * c:8 * c + 8, ssl])
    if hook2 is not None:
        hook2()
    if dbg and sq == 3:
        nc.sync.dma_start(dbg["xhd"][:], xh[:])
    for ct in range(6):
        dest, di = (QT, ct) if ct < 4 else (KT, ct - 4)
        pq = ps_qs.tile([128, W], F32, tag="qs", name=f"pq{sq}_{ct}")
        for dt_i in range(16):
            nc.tensor.matmul(pq[:], lhsT=wqk[:, ct, dt_i, :],
                             rhs=xh[:, dt_i, :],
                             start=(dt_i == 0), stop=(dt_i == 15))
        # RoPE: newE = E*c + swap(O*-s); newO = O*c + swap(E*s)
        t2 = rtmp.tile([128, W], BF16, tag="t2")
        nc.vector.tensor_mul(t2[:], pq[:], sn[:, ssl])
        pswp = ps_qs.tile([128, W], F32, tag="qs", name=f"sw{sq}_{ct}")
        nc.tensor.matmul(pswp[:], lhsT=P128t[:], rhs=t2[:],
                         start=True, stop=True)
        t1 = rtmp.tile([128, W], F32, tag="t1")
        nc.vector.tensor_mul(t1[:], pq[:], cs[:, ssl])
        nc.vector.tensor_add(dest[:, di, ssl], t1[:], pswp[:])
    for st in range(4):
        psv = ps_v.tile([128, 2, 128], F32, tag="psv")
        for dt_i in range(16):
            nc.tensor.matmul(psv[:], lhsT=xh[:, dt_i, st * 128:st * 128 + 128],
                             rhs=wv_all[:, dt_i, :],
                             start=(dt_i == 0), stop=(dt_i == 15))
        nc.vector.tensor_copy(Vn[:, sq * 4 + st, :, 0:128], psv[:])


def _scores_exp_tile(tc, ps_sc, ep, ets, QT, KT, ebias, qh, h, j, kt):
    """scores^T (keys on partitions) -> exp, one 128-key tile."""
    nc = tc.nc
    kvl, rho = h // 2, h % 2
    kof = rho * 1024 + kt * 128
    psc = ps_sc.tile([128, 1024], F32, tag="sc")
    for nch in range(2):
        nsl = slice(nch * 512, nch * 512 + 512)
        qsl = slice(qh * 1024 + nch * 512, qh * 1024 + nch * 512 + 512)
        nc.tensor.matmul(psc[:, nsl],
                         lhsT=KT[j * 64:(j + 1) * 64, kvl, kof:kof + 128],
                         rhs=QT[j * 64:(j + 1) * 64, h, qsl],
                         start=True, stop=True)
    et = ep.tile([128, 1024], ET_DT, tag="e")
    nc.scalar.activation(et[:], psc[:], AF.Exp, bias=ebias[:, 0:1],
                         scale=float(SCALE))
    ets.setdefault((h, j), [None] * 8)[kt] = et


def _scores_exp(tc, ps_sc, ep, ets, QT, KT, ebias, qh, h, j):
    for kt in range(8):
        _scores_exp_tile(tc, ps_sc, ep, ets, QT, KT, ebias, qh, h, j, kt)


def _pv_qt(tc, ps_pv, cpool, upool, ets, Vn, us, ms, dd, h, qt, lam):
    """pv both branches for one q-tile; u'' = pv1*d2 - lam*d1*pv2 + stats."""
    nc = tc.nc
    kvl, rho = h // 2, h % 2
    qsl = slice(qt * 128, qt * 128 + 128)
    ppv = ps_pv.tile([128, 2, 256], F32, tag="pv")  # bank-aligned halves
    for j in range(2):
        ej = ets[(h, j)]
        for kt in range(8):
            nc.tensor.matmul(ppv[:, j, 0:129],
                             lhsT=ej[kt][:, qsl],
                             rhs=Vn[:, rho * 8 + kt, kvl, :],
                             start=(kt == 0), stop=(kt == 7))
    # w2 = pv2 * (lam*d1);  dd = d1*d2;  u'' = pv1*d2 - w2
    w2 = cpool.tile([128, 128], F32, tag="w2")
    nc.vector.tensor_scalar(w2[:], ppv[:, 1, 0:128], ppv[:, 0, 128:129],
                            float(lam), op0=ALU.mult, op1=ALU.mult)
    nc.vector.tensor_scalar(dd[:, h % 2, h // 2, qt:qt + 1],
                            ppv[:, 0, 128:129], ppv[:, 1, 128:129],
                            None, op0=ALU.mult)
    u2 = upool.tile([128, 128], BF16, tag="u")
    nc.vector.scalar_tensor_tensor(u2[:], ppv[:, 0, 0:128],
                                   ppv[:, 1, 128:129], w2[:],
                                   op0=ALU.mult, op1=ALU.subtract)
    usq = cpool.tile([128, 128], BF16, tag="usq")
    nc.vector.tensor_mul(usq[:], u2[:], u2[:])
    nc.vector.tensor_reduce(ms[:, h % 2, h // 2, qt:qt + 1], usq[:],
                            op=ALU.add, axis=mybir.AxisListType.X)
    us[(h, qt)] = u2


def _norm_col(tc, ps_tr, cpool, us, rs, identt, ustT, qh, h, qt):
    """ust = u'' * rs * sqrt(128); transpose into wo-lhsT layout."""
    nc = tc.nc
    # sqrt(128) restores the mean-vs-sum normalization of x'
    ust = cpool.tile([128, 128], BF16, tag="ust")
    nc.vector.tensor_scalar(ust[:], us[(h, qt)][:],
                            rs[:, h % 2, h // 2, qt:qt + 1],
                            float(math.sqrt(128.0)),
                            op0=ALU.mult, op1=ALU.mult)
    pst = ps_tr.tile([128, 128], BF16, tag="tr")
    nc.tensor.transpose(pst[:], ust[:], identt[:])
    nc.vector.tensor_copy(ustT[:, h, qh * 1024 + qt * 128:
                               qh * 1024 + qt * 128 + 128], pst[:])


def _wo_st(tc, ps_o, obp, ustT, wot, out, qh, st):
    """row-parallel wo for one s-tile; single batched bf16 store."""
    nc = tc.nc
    sof = (qh * 8 + st) * 128
    ob = obp.tile([128, 2048], OUT_DT, tag="ob")
    for ech in range(4):
        po3 = ps_o.tile([128, 2, 256], F32, tag="pv", name="po")
        po = po3.rearrange("p a b -> p (a b)")
        for r in range(4):
            nc.tensor.matmul(po[:], lhsT=ustT[:, r, sof:sof + 128],
                             rhs=wot[:, r, ech * 512:ech * 512 + 512],
                             start=(r == 0), stop=(r == 3))
        osl = slice(ech * 512, ech * 512 + 512)
        if ech % 2 == 0:
            nc.scalar.activation(ob[:, osl], po[:], AF.Copy, bias=0.0,
                                 scale=1.0)
        else:
            nc.vector.tensor_copy(ob[:, osl], po[:])
    nc.sync.dma_start(out[sof:sof + 128, :], ob[:])


# ---------------------------------------------------------------- host side

_PERM64 = np.concatenate([np.arange(0, 64, 2), np.arange(1, 64, 2)])


def make_core_inputs(core, x, wq, wk, wv, wo, subln_w, lambda_init,
                     freqs_cos, freqs_sin):
    b, g = divmod(core, 4)
    bf = ml_dtypes.bfloat16
    qcols = np.empty(512, np.int64)
    for hl in range(4):
        for j in range(2):
            qcols[hl * 128 + j * 64:hl * 128 + j * 64 + 64] = \
                ((4 * g + hl) * 2 + j) * 64 + _PERM64
    kcols = np.empty(256, np.int64)
    for kvl in range(2):
        for j in range(2):
            kcols[kvl * 128 + j * 64:kvl * 128 + j * 64 + 64] = \
                ((2 * g + kvl) * 2 + j) * 64 + _PERM64
    vcols = np.arange(256) + 2 * g * 128

    # wq/wk packed per column-tile: [6, 128(part), 16*128] so every DMA
    # descriptor is one contiguous 4KB run per partition.
    wq_c = wq[:, qcols].astype(np.float32)   # [2048, 512]
    wk_c = wk[:, kcols].astype(np.float32)   # [2048, 256]
    wqk = np.empty((6, 128, 2048), np.float32)
    for ct in range(4):
        wqk[ct] = wq_c[:, ct * 128:(ct + 1) * 128].reshape(
            16, 128, 128).transpose(1, 0, 2).reshape(128, 2048)
    for ct in range(2):
        wqk[4 + ct] = wk_c[:, ct * 128:(ct + 1) * 128].reshape(
            16, 128, 128).transpose(1, 0, 2).reshape(128, 2048)

    cosT = np.ascontiguousarray(freqs_cos.T.astype(np.float32))  # [32, S]
    sinT = np.ascontiguousarray(freqs_sin.T.astype(np.float32))
    wo_s = wo[512 * g: 512 * g + 512, :].astype(np.float32).copy()
    wo_s *= np.tile(subln_w.astype(np.float32)
                    * (1.0 - np.float32(np.asarray(lambda_init)[0])), 4)[:, None]

    swap = np.empty(128, np.int64)
    for blk in range(4):
        swap[blk * 32:blk * 32 + 32] = \
            (blk + 1 if blk % 2 == 0 else blk - 1) * 32 + np.arange(32)
    P = np.zeros((128, 128), np.float32)
    P[swap, np.arange(128)] = 1.0   # P[p, r] = 1 iff p == swap(r)

    return {
        "xT": np.ascontiguousarray(x[b].T.astype(np.float32)).astype(bf),
        "wqk_s": wqk.astype(bf),
        "wv_s": np.ascontiguousarray(wv[:, vcols].astype(np.float32)).astype(bf),
        "wo_s": wo_s.astype(bf),
        "cs128": np.tile(cosT, (4, 1)).astype(bf),
        "sn128": np.concatenate([sinT, -sinT, sinT, -sinT], axis=0).astype(bf),
        "P128": P.astype(bf),
        "ident": np.eye(128, dtype=np.float32).astype(bf),
    }


def compute_lambda(lambda_q1, lambda_k1, lambda_q2, lambda_k2, lambda_init):
    l1 = np.exp(np.sum(np.float32(lambda_q1) * np.float32(lambda_k1),
                       dtype=np.float32))
    l2 = np.exp(np.sum(np.float32(lambda_q2) * np.float32(lambda_k2),
                       dtype=np.float32))
    return float(l1 - l2 + np.float32(np.asarray(lambda_init)[0]))


def kernel(x, wq, wk, wv, wo, lambda_q1, lambda_k1, lambda_q2, lambda_k2,
           lambda_init, subln_w, freqs_cos, freqs_sin):
    global LAST_RESULTS
    x = np.asarray(x); wq = np.asarray(wq); wk = np.asarray(wk)
    wv = np.asarray(wv); wo = np.asarray(wo)
    lam = compute_lambda(lambda_q1, lambda_k1, lambda_q2, lambda_k2, lambda_init)

    nc = build_program(lam)
    in_maps = [make_core_inputs(c, x, wq, wk, wv, wo,
                                np.asarray(subln_w), np.asarray(lambda_init),
                                np.asarray(freqs_cos), np.asarray(freqs_sin))
               for c in range(NCORES)]
    res = run_bass_kernel_spmd(nc, in_maps, list(range(NCORES)), trace=TRACE)
    LAST_RESULTS = res
    outs = [np.asarray(res.results[c]["out"]).astype(np.float32)
            for c in range(NCORES)]
    full = np.empty((B, S, DIM), np.float32)
    for b in range(B):
        full[b] = outs[4 * b] + outs[4 * b + 1] + outs[4 * b + 2] + outs[4 * b + 3]
    return full
